# revision 1
# baseline (speedup 1.0000x reference)
"""MultiHeadAttention Trainium2 kernel, 8-way tensor-parallel over heads.

B=4, T=2048, C=1024, H=16 heads, Dh=64. Each of the 8 NeuronCores owns 2
heads: it computes Q^T/K^T (as [2*Dh, T] per batch), V (as [T, 2*Dh] via PE
transpose), attention in the transposed layout (S^T = K_tile^T @ Q^T with the
contraction over Dh; exp on ScalarE; P@V as V_ones^T @ E^T which also yields
the softmax denominator from an appended ones column), and a partial output
projection against its 128 columns of W_out. The host sums the 8 partials
(fp64) to unshard.

All matmuls run as float32r (full-rate fp32, ~1e-4 relative accuracy).
"""
import sys
sys.path.insert(0, '/opt/trn_rl_repo')
import numpy as np

import concourse.bass as bass
import concourse.mybir as mybir
import concourse.tile as tile
from concourse import bacc
from concourse.bass_utils import run_bass_kernel_spmd
from concourse.masks import make_identity

F32 = mybir.dt.float32
F32R = mybir.dt.float32r
AF = mybir.ActivationFunctionType

B, T, C = 4, 2048, 1024
H, DH = 16, 64
NCORES = 8
HPC = H // NCORES          # heads per core (2)
D2 = HPC * DH              # 128, local concat dim
BT = B * T                 # 8192
NT = T // 512              # q/t tiles of 512 per batch (4)
NK = T // 128              # k tiles of 128 per batch (16)
CCH = C // 128             # contraction chunks (8)

_NC_CACHE = {}


def build_nc():
    nc = bacc.Bacc()

    xp = nc.dram_tensor("xp", [128, B * NT, CCH, 512], F32, kind="ExternalInput")
    wq = nc.dram_tensor("wq", [128, CCH, D2], F32, kind="ExternalInput")
    wk = nc.dram_tensor("wk", [128, CCH, D2], F32, kind="ExternalInput")
    wv = nc.dram_tensor("wv", [128, CCH, D2], F32, kind="ExternalInput")
    bq = nc.dram_tensor("bq", [D2, 1], F32, kind="ExternalInput")
    bk = nc.dram_tensor("bk", [D2, 1], F32, kind="ExternalInput")
    bv = nc.dram_tensor("bv", [D2, 1], F32, kind="ExternalInput")
    wo = nc.dram_tensor("wo", [128, C], F32, kind="ExternalInput")
    bo = nc.dram_tensor("bo", [128, C], F32, kind="ExternalInput")
    y = nc.dram_tensor("y", [BT, C], F32, kind="ExternalOutput")

    with tile.TileContext(nc) as tc:
        with (
            tc.tile_pool(name="singles", bufs=1) as singles,
            tc.tile_pool(name="xin", bufs=3) as xin,
            tc.tile_pool(name="qkv", bufs=2) as qkv,
            tc.tile_pool(name="vtmp", bufs=2) as vtmp_pool,
            tc.tile_pool(name="esb", bufs=4) as esb,
            tc.tile_pool(name="rsb", bufs=2) as rsb,
            tc.tile_pool(name="osb", bufs=2) as osb,
            tc.tile_pool(name="outsb", bufs=3) as outsb,
            # 8 PSUM banks total: s2 2x2 + pv 2x1 + po 2x1
            tc.tile_pool(name="s2_ps", bufs=2, space="PSUM") as s2_ps,
            tc.tile_pool(name="small_ps", bufs=4, space="PSUM") as small_ps,
        ):
            ident = singles.tile([128, 128], F32)
            make_identity(nc, ident)
            warm_f = singles.tile([128, 512], F32, tag="warm_f")
            nc.vector.memset(warm_f, 1.0)
            warm_r = singles.tile([128, 512], F32R, tag="warm_r")
            nc.vector.tensor_copy(out=warm_r, in_=warm_f)
            for wi in range(12):
                wps = small_ps.tile([128, 512], F32, tag="sm", name=f"warm{wi}")
                nc.tensor.matmul(out=wps, lhsT=warm_r[:, 0:128], rhs=warm_r,
                                 start=True, stop=True)
            ones16 = singles.tile([128, NK, 1], F32)
            nc.vector.memset(ones16, 1.0)

            wq_sb = singles.tile([128, CCH, D2], F32R, tag="wq")
            wk_sb = singles.tile([128, CCH, D2], F32R, tag="wk")
            wv_sb = singles.tile([128, CCH, D2], F32R, tag="wv")
            for w_dram, w_sb in ((wq, wq_sb), (wk, wk_sb), (wv, wv_sb)):
                nc.sync.dma_start(out=w_sb, in_=w_dram[:, :, :].bitcast(F32R))
            bq_sb = singles.tile([D2, 1], F32, tag="bq")
            bk_sb = singles.tile([D2, 1], F32, tag="bk")
            bv_sb = singles.tile([D2, 1], F32, tag="bv")
            nc.sync.dma_start(out=bq_sb, in_=bq[:, :])
            nc.sync.dma_start(out=bk_sb, in_=bk[:, :])
            nc.sync.dma_start(out=bv_sb, in_=bv[:, :])
            wo_sb = singles.tile([128, C], F32R, tag="wo")
            nc.sync.dma_start(out=wo_sb, in_=wo[:, :].bitcast(F32R))
            bo_sb = singles.tile([128, C], F32, tag="bo")
            nc.sync.dma_start(out=bo_sb, in_=bo[:, :])

            for b in range(B):
                qT = qkv.tile([D2, T], F32R, tag="q")
                kT = qkv.tile([D2, T], F32R, tag="k")
                # per k-tile lhsT layout (193 cols):
                #   h0: cols 0:65   = [V_h0 | 1]            (M=65:  num@0:64, Z@64)
                #   h1: cols 65:193 = [junk32 | 1 | junk31 | V_h1] (M=128: Z@32,
                #        num@64:128; junk columns make junk PSUM rows, never read)
                v1 = qkv.tile([128, NK, 193], F32R, tag="v")
                nc.vector.tensor_copy(out=v1[:, :, DH:DH + 1], in_=ones16)
                nc.vector.tensor_copy(out=v1[:, :, 97:98], in_=ones16)

                # ---- QKV projection for batch b ----
                for tt in range(NT):
                    t0 = tt * 512
                    xt = xin.tile([128, CCH, 512], F32R)
                    nc.sync.dma_start(
                        out=xt,
                        in_=xp[:, b * NT + tt, :, :].bitcast(F32R))
                    for w_sb, b_sb, dest in ((wq_sb, bq_sb, qT), (wk_sb, bk_sb, kT)):
                        ps = small_ps.tile([128, 512], F32, tag="sm")
                        for ci in range(CCH):
                            nc.tensor.matmul(out=ps, lhsT=w_sb[:, ci, :],
                                             rhs=xt[:, ci, :],
                                             start=(ci == 0), stop=(ci == CCH - 1))
                        nc.vector.tensor_scalar_add(out=dest[:, t0:t0 + 512],
                                                    in0=ps, scalar1=b_sb)
                    # V^T, then transpose into [t, d] layout
                    ps = small_ps.tile([128, 512], F32, tag="sm")
                    for ci in range(CCH):
                        nc.tensor.matmul(out=ps, lhsT=wv_sb[:, ci, :],
                                         rhs=xt[:, ci, :],
                                         start=(ci == 0), stop=(ci == CCH - 1))
                    vt = vtmp_pool.tile([128, 512], F32)
                    nc.vector.tensor_scalar_add(out=vt, in0=ps, scalar1=bv_sb)
                    for s in range(4):
                        tp = small_ps.tile([128, 512], F32, tag="sm")
                        nc.tensor.transpose(out=tp[:, 0:128],
                                            in_=vt[:, s * 128:(s + 1) * 128],
                                            identity=ident)
                        kt = tt * 4 + s
                        sl = v1[:, kt, :]
                        dst = bass.AP(tensor=sl.tensor, offset=sl.offset,
                                      ap=[list(sl.ap[0]), [129, 2], [1, DH]])
                        nc.vector.tensor_copy(
                            out=dst,
                            in_=tp[:, 0:128].rearrange("p (g x) -> p g x", g=2))

                # ---- attention for batch b (both heads interleaved; the
                # S(kt) matmuls are issued before PV(kt-1) so the in-order PE
                # queue never stalls behind the exp of the current kt) ----
                oT2 = osb.tile([128, T], F32R, tag="o2")
                for qt in range(NT):
                    q0 = qt * 512
                    pv0 = small_ps.tile([DH + 1, 512], F32, tag="sm")
                    pv1 = small_ps.tile([128, 512], F32, tag="sm")
                    pvs = [pv0, pv1]
                    lh = [(0, DH + 1), (DH + 1, 193)]
                    ets = []
                    LA = 2  # PV lookahead: PV(kt-LA) issues with S(kt)
                    for kt in range(NK):
                        s2 = s2_ps.tile([128, 1024], F32, tag="s2")
                        for h in range(HPC):
                            hs = h * DH
                            nc.tensor.matmul(
                                out=s2[:, h * 512:(h + 1) * 512],
                                lhsT=kT[hs:hs + DH, kt * 128:(kt + 1) * 128],
                                rhs=qT[hs:hs + DH, q0:q0 + 512],
                                start=True, stop=True)
                        if kt >= LA:
                            etp = ets[kt - LA]
                            for h in range(HPC):
                                nc.tensor.matmul(
                                    out=pvs[h],
                                    lhsT=v1[:, kt - LA, lh[h][0]:lh[h][1]],
                                    rhs=etp[:, h * 512:(h + 1) * 512],
                                    start=(kt - LA == 0), stop=False)
                        et = esb.tile([128, 1024], F32R)
                        nc.scalar.activation(out=et, in_=s2,
                                             func=AF.Exp, scale=0.125)
                        ets.append(et)
                    for kt in range(NK - LA, NK):
                        for h in range(HPC):
                            nc.tensor.matmul(
                                out=pvs[h],
                                lhsT=v1[:, kt, lh[h][0]:lh[h][1]],
                                rhs=ets[kt][:, h * 512:(h + 1) * 512],
                                start=False, stop=(kt == NK - 1))
                    # evacuate each PV accumulator to SBUF with one copy (frees
                    # the PSUM slot immediately); normalize from the copy.
                    # h0: num@[0:64], Z@[64]; h1: num@[64:128], Z@[32]
                    for h, pv, zrow, o_lo, o_hi in (
                            (0, pv0, DH, 0, DH),
                            (1, pv1, 32, DH, 128)):
                        pvc = rsb.tile([128, 512], F32, tag="pvc")
                        if h == 0:
                            nc.vector.tensor_copy(out=pvc[0:DH + 1, :],
                                                  in_=pv[0:DH + 1, :])
                        else:
                            nc.vector.tensor_copy(out=pvc[32:33, :],
                                                  in_=pv[32:33, :])
                            nc.vector.tensor_copy(out=pvc[DH:128, :],
                                                  in_=pv[DH:128, :])
                        z = rsb.tile([1, 512], F32, tag="z")
                        nc.vector.tensor_copy(out=z, in_=pvc[zrow:zrow + 1, :])
                        r = rsb.tile([1, 512], F32, tag="r")
                        nc.vector.reciprocal_approx_fast(out=r, in_=z)
                        rbc = rsb.tile([128, 512], F32, tag="rbc")
                        nc.gpsimd.partition_broadcast(rbc[0:o_hi, :], r)
                        nc.vector.tensor_mul(out=oT2[o_lo:o_hi, q0:q0 + 512],
                                             in0=pvc[o_lo:o_hi, :],
                                             in1=rbc[o_lo:o_hi, :])

                # ---- partial out-projection for batch b ----
                for ts in range(T // 128):
                    ot = outsb.tile([128, C], F32)
                    for n in range(2):
                        n0 = n * 512
                        ops = small_ps.tile([128, 512], F32, tag="sm")
                        nc.tensor.matmul(
                            out=ops,
                            lhsT=oT2[:, ts * 128:(ts + 1) * 128],
                            rhs=wo_sb[:, n0:n0 + 512],
                            start=True, stop=True)
                        nc.vector.tensor_add(out=ot[:, n0:n0 + 512],
                                             in0=ops, in1=bo_sb[:, n0:n0 + 512])
                    nc.sync.dma_start(
                        out=y[b * T + ts * 128:b * T + (ts + 1) * 128, :],
                        in_=ot)

    nc.compile()
    return nc


def make_in_maps(x, W_qkv, b_qkv, W_out, b_out):
    # x pre-tiled to the exact SBUF layout: xp[p, tile, ci, c] = x[tile*512+c, ci*128+p]
    xp = np.ascontiguousarray(
        x.reshape(B * NT, 512, CCH, 128).transpose(3, 0, 2, 1))
    bo = np.ascontiguousarray(
        np.broadcast_to(b_out.astype(np.float32) / NCORES, (128, C)))
    in_maps = []
    for c in range(NCORES):
        r0 = c * D2
        def wshuf(wslice):
            # [D2, C] weight rows -> lhsT chunks [128 p, CCH, D2]
            return np.ascontiguousarray(
                wslice.T.reshape(CCH, 128, D2).transpose(1, 0, 2))
        wq = wshuf(W_qkv[r0:r0 + D2, :])
        wk = wshuf(W_qkv[C + r0:C + r0 + D2, :])
        wv = wshuf(W_qkv[2 * C + r0:2 * C + r0 + D2, :])
        bqc = np.ascontiguousarray(b_qkv[r0:r0 + D2].reshape(D2, 1))
        bkc = np.ascontiguousarray(b_qkv[C + r0:C + r0 + D2].reshape(D2, 1))
        bvc = np.ascontiguousarray(b_qkv[2 * C + r0:2 * C + r0 + D2].reshape(D2, 1))
        woc = np.ascontiguousarray(W_out[:, r0:r0 + D2].T)
        in_maps.append({
            "xp": xp, "wq": wq, "wk": wk, "wv": wv,
            "bq": bqc, "bk": bkc, "bv": bvc, "wo": woc, "bo": bo,
        })
    return in_maps


def run(x, W_qkv, b_qkv, W_out, b_out, trace=False):
    if "nc" not in _NC_CACHE:
        _NC_CACHE["nc"] = build_nc()
    nc = _NC_CACHE["nc"]
    in_maps = make_in_maps(
        np.asarray(x, dtype=np.float32), np.asarray(W_qkv, dtype=np.float32),
        np.asarray(b_qkv, dtype=np.float32), np.asarray(W_out, dtype=np.float32),
        np.asarray(b_out, dtype=np.float32))
    res = run_bass_kernel_spmd(nc, in_maps, core_ids=list(range(NCORES)),
                               trace=trace)
    acc = np.zeros((BT, C), dtype=np.float64)
    for c in range(NCORES):
        acc += res.results[c]["y"]
    out = acc.astype(np.float32).reshape(B, T, C)
    return out, res


def kernel(x, W_qkv, b_qkv, W_out, b_out):
    out, _ = run(x, W_qkv, b_qkv, W_out, b_out, trace=False)
    return out



# revision 23
# speedup vs baseline: 1.1130x; 1.1130x over previous
"""MultiHeadAttention Trainium2 kernel, 8-way tensor-parallel over heads.

B=4, T=2048, C=1024, H=16 heads, Dh=64. Each of the 8 NeuronCores owns 2
heads. v3 design:

- x and the QKV weights ship as bf16 (weights pre-scaled by 16 on the host);
  Q/K/V projections are plain bf16 matmuls.
- Q and K activations are quantized to fp8e4m3 on the Vector engine
  (K as a compensated hi+lo pair). S^T = K^T Q runs as one fp8 DoubleRow
  matmul per (kt, head): pair slots = (K_hi, K_lo) against duplicated Q
  -> 2x PE rate with compensated K.
- exp runs on the Scalar engine (true Exp, bf16 out) for most k-tiles; a
  per-q-tile subset runs on the Vector engine via a Schraudolph-style affine
  map straight into bf16 bit patterns (int16 bitcast). The last q-tile of
  each batch keeps exp fully on ACT because the Vector engine is busy
  quantizing the next batch's QKV.
- PV runs in bf16 (fp8 E fails the error budget: softmax weight noise does
  not cancel in the PV average). The ones column (value 16) provides the
  softmax denominator; V is 16x-scaled so the 16s cancel.
- The QKV phase of batch b+1 is interleaved into the last q-tile of batch b
  so the PE never idles while exp drains.
- Partial out-projection in float32r; partials are stored as bf16 and summed
  (plus bias) on the host in fp64.
"""
import sys
sys.path.insert(0, '/opt/trn_rl_repo')
import numpy as np
import ml_dtypes

import concourse.bass as bass
import concourse.mybir as mybir
import concourse.tile as tile
from concourse import bacc
from concourse.bass_utils import run_bass_kernel_spmd
from concourse.masks import make_identity

F32 = mybir.dt.float32
F32R = mybir.dt.float32r
BF16 = mybir.dt.bfloat16
F8 = mybir.dt.float8e4
I16 = mybir.dt.int16
AF = mybir.ActivationFunctionType
ALU = mybir.AluOpType
DR = mybir.MatmulPerfMode.DoubleRow

B, T, C = 4, 2048, 1024
H, DH = 16, 64
NCORES = 8
HPC = H // NCORES          # heads per core (2)
D2 = HPC * DH              # 128, local concat dim
BT = B * T                 # 8192
NT = T // 512              # q/t tiles of 512 per batch (4)
NK = T // 128              # k tiles of 128 per batch (16)
CCH = C // 128             # contraction chunks (8)

LN2 = float(np.log(2.0))
SCALE = 1.0 / 2048.0       # exp scale on S psum (1/(sqrt(Dh)*16*16))
SCHR_A = 128.0 / LN2 / 2048.0          # Schraudolph multiplier on S psum
SCHR_C = -11.0                          # tuned offset (trunc rounding)
SCHR_B = 127.0 * 128.0 + SCHR_C
# k-tiles whose exp runs on DVE, per q-tile position. qt==3 stays on ACT
# (DVE is busy quantizing the next batch's QKV there).
SCHR_KT_EARLY = (6, 8, 10, 12, 14)      # batches that also quantize next QKV
SCHR_KT_LAST = (2, 4, 6, 8, 10, 12, 14)  # final batch: DVE has no quant work
LA = 2                     # PV lookahead behind S in the PE stream
USE_DMA_T = False           # V transpose via DMA xbar (False: PE transpose)
USE_SCHR = False            # Schraudolph exp tiles on DVE
USE_SDR = False             # S matmul via fp8 DoubleRow (False: 2 plain fp8)

E4M3 = ml_dtypes.float8_e4m3
BF16NP = ml_dtypes.bfloat16

_NC_CACHE = {}


def build_nc():
    nc = bacc.Bacc()

    xb = nc.dram_tensor("xb", [128, B * NT, CCH, 512], BF16, kind="ExternalInput")
    wqb = nc.dram_tensor("wqb", [128, CCH, 128], BF16, kind="ExternalInput")
    wkb = nc.dram_tensor("wkb", [128, CCH, 128], BF16, kind="ExternalInput")
    wvb = nc.dram_tensor("wvb", [128, CCH, 128], BF16, kind="ExternalInput")
    bq = nc.dram_tensor("bq", [D2, 1], F32, kind="ExternalInput")
    bk = nc.dram_tensor("bk", [D2, 1], F32, kind="ExternalInput")
    bv = nc.dram_tensor("bv", [D2, 1], F32, kind="ExternalInput")
    wo = nc.dram_tensor("wo", [128, C], F32, kind="ExternalInput")
    y = nc.dram_tensor("y", [BT, C], BF16, kind="ExternalOutput")

    with tile.TileContext(nc) as tc:
        with (
            tc.tile_pool(name="singles", bufs=1) as singles,
            tc.tile_pool(name="xin", bufs=3) as xin,
            tc.tile_pool(name="qk", bufs=2) as qk,
            tc.tile_pool(name="v1p", bufs=2) as v1p,
            tc.tile_pool(name="vtp", bufs=2) as vtp,
            tc.tile_pool(name="esb", bufs=2) as esb,
            tc.tile_pool(name="osb", bufs=3) as osb,
            tc.tile_pool(name="outsb", bufs=2) as outsb,
            tc.tile_pool(name="rsb", bufs=3) as rsb,
            tc.tile_pool(name="s2_ps", bufs=3, space="PSUM") as s2_ps,
            tc.tile_pool(name="pv_ps", bufs=2, space="PSUM") as pv_ps,
        ):
            ident = singles.tile([128, 128], BF16)
            wq_sb = singles.tile([128, CCH, 128], BF16, tag="wq")
            wk_sb = singles.tile([128, CCH, 128], BF16, tag="wk")
            wv_sb = singles.tile([128, CCH, 128], BF16, tag="wv")
            for w_dram, w_sb in ((wqb, wq_sb), (wkb, wk_sb), (wvb, wv_sb)):
                nc.sync.dma_start(out=w_sb, in_=w_dram[:, :, :])
            bq_sb = singles.tile([D2, 1], F32, tag="bq")
            bk_sb = singles.tile([D2, 1], F32, tag="bk")
            bv_sb = singles.tile([D2, 1], F32, tag="bv")
            nc.sync.dma_start(out=bq_sb, in_=bq[:, :])
            nc.sync.dma_start(out=bk_sb, in_=bk[:, :])
            nc.sync.dma_start(out=bv_sb, in_=bv[:, :])
            wo_sb = singles.tile([128, C], F32R, tag="wo")
            nc.sync.dma_start(out=wo_sb, in_=wo[:, :].bitcast(F32R))

            make_identity(nc, ident)
            warm_f = singles.tile([128, 512], F32, tag="warm_f")
            nc.vector.memset(warm_f, 1.0)
            warm_r = singles.tile([128, 512], F32R, tag="warm_r")
            nc.vector.tensor_copy(out=warm_r, in_=warm_f)
            for wi in range(20):
                wps = s2_ps.tile([128, 1024], F32, tag="s2", name=f"warm{wi}")
                nc.tensor.matmul(out=wps[:, 0:512], lhsT=warm_r[:, 0:128],
                                 rhs=warm_r, start=True, stop=True)

            st = {}      # per-batch activation tiles

            def emit_A_tt(b, tt):
                if tt == 0:
                    qT8 = qk.tile([128, 2, T], F8, tag="q")
                    kHL = qk.tile([128, 2, T], F8, tag="k")
                    v1 = v1p.tile([128, NK, 2, 65], BF16, tag="v1")
                    # col 64 of each head block = ones(16) for the softmax
                    # denominator (V is 16x-scaled, so the 16s cancel).
                    nc.vector.memset(v1[:, :, :, 64:65], 16.0)
                    oT2 = osb.tile([128, T], F32R, tag="o2")
                    st[b] = (qT8, kHL, v1, oT2)
                qT8, kHL, v1, oT2 = st[b]
                t0 = tt * 512
                xt = xin.tile([128, CCH, 512], BF16)
                nc.sync.dma_start(out=xt, in_=xb[:, b * NT + tt])
                # Q and K share one 2-bank psum tile (halves)
                qkps = s2_ps.tile([128, 1024], F32, tag="s2")
                qps = qkps[:, 0:512]
                kps = qkps[:, 512:1024]
                for ci in range(CCH):
                    nc.tensor.matmul(out=qps, lhsT=wq_sb[:, ci], rhs=xt[:, ci],
                                     start=(ci == 0), stop=(ci == CCH - 1))
                for ci in range(CCH):
                    nc.tensor.matmul(out=kps, lhsT=wk_sb[:, ci], rhs=xt[:, ci],
                                     start=(ci == 0), stop=(ci == CCH - 1))
                for i in range(2):
                    nc.vector.tensor_scalar_add(
                        out=qT8[:, i, t0:t0 + 512], in0=qps, scalar1=bq_sb)
                nc.vector.tensor_scalar_add(
                    out=kHL[:, 0, t0:t0 + 512], in0=kps, scalar1=bk_sb)
                nc.vector.scalar_tensor_tensor(
                    out=kHL[:, 1, t0:t0 + 512], in0=kps, scalar=bk_sb,
                    in1=kHL[:, 0, t0:t0 + 512], op0=ALU.add, op1=ALU.subtract)
                # V (+ transpose into [tok, feat])
                vtile = s2_ps.tile([128, 1024], F32, tag="s2")
                vps = vtile[:, 0:512]
                for ci in range(CCH):
                    nc.tensor.matmul(out=vps, lhsT=wv_sb[:, ci], rhs=xt[:, ci],
                                     start=(ci == 0), stop=(ci == CCH - 1))
                vt = vtp.tile([128, 512], BF16)
                nc.vector.tensor_scalar_add(out=vt, in0=vps, scalar1=bv_sb)
                if USE_DMA_T:
                    for h in range(HPC):
                        nc.sync.dma_start_transpose(
                            out=v1[:, tt * 4:(tt + 1) * 4, h, 0:64],
                            in_=vt[h * 64:(h + 1) * 64, :])
                else:
                    tp = s2_ps.tile([128, 1024], BF16, tag="s2")
                    for s in range(4):
                        nc.tensor.transpose(out=tp[:, s * 128:(s + 1) * 128],
                                            in_=vt[:, s * 128:(s + 1) * 128],
                                            identity=ident)
                    dstv = v1[:, tt * 4:(tt + 1) * 4, :, 0:64]
                    srcv = tp[:, 0:512].rearrange("p (s h d) -> p s h d",
                                                  s=4, h=2)
                    nc.vector.tensor_copy(out=dstv, in_=srcv)

            def emit_outproj(b, qt):
                _, _, _, oT2 = st[b]
                ot = outsb.tile([128, 4, C], BF16, tag="ot")
                for ts in range(4):
                    tcol = qt * 512 + ts * 128
                    ops = s2_ps.tile([128, 1024], F32, tag="s2")
                    for n in range(2):
                        n0 = n * 512
                        nc.tensor.matmul(out=ops[:, n0:n0 + 512],
                                         lhsT=oT2[:, tcol:tcol + 128],
                                         rhs=wo_sb[:, n0:n0 + 512],
                                         start=True, stop=True)
                    if ts % 2 == 1:
                        nc.scalar.copy(out=ot[:, ts, :], in_=ops)
                    else:
                        nc.vector.tensor_copy(out=ot[:, ts, :], in_=ops)
                row0 = b * T + qt * 512
                ydst = bass.AP(tensor=y, offset=row0 * C,
                               ap=[[C, 128], [128 * C, 4], [1, C]])
                nc.sync.dma_start(out=ydst, in_=ot)

            def emit_B(b, qt):
                qT8, kHL, v1, oT2 = st[b]
                q0 = qt * 512
                schr = SCHR_KT_LAST if b == B - 1 else SCHR_KT_EARLY
                e = esb.tile([128, NK, 1024], BF16, tag="e")
                pv0 = pv_ps.tile([128, 512], F32, tag="pv")
                pv1 = pv_ps.tile([128, 512], F32, tag="pv")
                pvs = (pv0, pv1)

                def pv_mm(kt):
                    for h in range(HPC):
                        nc.tensor.matmul(
                            out=pvs[h][0:65, :],
                            lhsT=v1[:, kt, h, :],
                            rhs=e[:, kt, h * 512:(h + 1) * 512],
                            start=(kt == 0), stop=(kt == NK - 1))

                for kt in range(NK):
                    s2 = s2_ps.tile([128, 1024], F32, tag="s2")
                    for h in range(HPC):
                        hs = h * DH
                        if USE_SDR:
                            nc.tensor.matmul(
                                out=s2[:, h * 512:(h + 1) * 512],
                                lhsT=kHL[hs:hs + DH, :, kt * 128:(kt + 1) * 128],
                                rhs=qT8[hs:hs + DH, :, q0:q0 + 512],
                                start=True, stop=True, perf_mode=DR)
                        else:
                            for i in range(2):
                                nc.tensor.matmul(
                                    out=s2[:, h * 512:(h + 1) * 512],
                                    lhsT=kHL[hs:hs + DH, i, kt * 128:(kt + 1) * 128],
                                    rhs=qT8[hs:hs + DH, 0, q0:q0 + 512],
                                    start=(i == 0), stop=(i == 1))
                    if USE_SCHR and kt in schr:
                        nc.vector.tensor_scalar(
                            out=e[:, kt, :].bitcast(I16), in0=s2,
                            scalar1=SCHR_A, scalar2=SCHR_B,
                            op0=ALU.mult, op1=ALU.add)
                    else:
                        nc.scalar.activation(out=e[:, kt, :], in_=s2,
                                             func=AF.Exp, scale=SCALE)
                    if kt >= LA:
                        pv_mm(kt - LA)
                    # interleave next batch's QKV (one tile per q-tile)
                    if kt == 7 and b + 1 < B:
                        emit_A_tt(b + 1, qt)
                for kt in range(NK - LA, NK):
                    pv_mm(kt)

                for h, pv in enumerate(pvs):
                    z = rsb.tile([1, 512], F32, tag="z")
                    nc.vector.tensor_copy(out=z, in_=pv[64:65, :])
                    r = rsb.tile([1, 512], F32, tag="r")
                    nc.vector.reciprocal_approx_fast(out=r, in_=z)
                    rbc = rsb.tile([128, 512], F32, tag="rbc")
                    nc.gpsimd.partition_broadcast(rbc[0:DH, :], r)
                    nc.vector.tensor_tensor(
                        out=oT2[h * DH:(h + 1) * DH, q0:q0 + 512],
                        in0=pv[0:DH, :], in1=rbc[0:DH, :], op=ALU.mult)

            for tt in range(NT):
                emit_A_tt(0, tt)
            pending = []
            for g in range(B * NT):
                b, qt = divmod(g, NT)
                emit_B(b, qt)
                pending.append((b, qt))
                if len(pending) >= 2:
                    emit_outproj(*pending.pop(0))
            for bq_ in pending:
                emit_outproj(*bq_)

    nc.compile()
    return nc


def make_in_maps(x, W_qkv, b_qkv, W_out, b_out):
    xf = np.ascontiguousarray(
        x.reshape(B * NT, 512, CCH, 128).transpose(3, 0, 2, 1)).astype(np.float32)
    xbm = np.ascontiguousarray(xf.astype(BF16NP))   # [128, 16, CCH, 512]

    def wprep(wslice):
        w16 = (16.0 * wslice).astype(np.float32).astype(BF16NP)
        # [D2, C] -> [128p, CCH, D2]
        return np.ascontiguousarray(
            w16.T.reshape(CCH, 128, D2).transpose(1, 0, 2))

    in_maps = []
    for c in range(NCORES):
        r0 = c * D2
        in_maps.append({
            "xb": xbm,
            "wqb": wprep(W_qkv[r0:r0 + D2, :]),
            "wkb": wprep(W_qkv[C + r0:C + r0 + D2, :]),
            "wvb": wprep(W_qkv[2 * C + r0:2 * C + r0 + D2, :]),
            "bq": np.ascontiguousarray(
                16.0 * b_qkv[r0:r0 + D2].reshape(D2, 1)).astype(np.float32),
            "bk": np.ascontiguousarray(
                16.0 * b_qkv[C + r0:C + r0 + D2].reshape(D2, 1)).astype(np.float32),
            "bv": np.ascontiguousarray(
                16.0 * b_qkv[2 * C + r0:2 * C + r0 + D2].reshape(D2, 1)).astype(np.float32),
            "wo": np.ascontiguousarray(W_out[:, r0:r0 + D2].T).astype(np.float32),
        })
    return in_maps


def run(x, W_qkv, b_qkv, W_out, b_out, trace=False):
    if "nc" not in _NC_CACHE:
        _NC_CACHE["nc"] = build_nc()
    nc = _NC_CACHE["nc"]
    in_maps = make_in_maps(
        np.asarray(x, dtype=np.float32), np.asarray(W_qkv, dtype=np.float32),
        np.asarray(b_qkv, dtype=np.float32), np.asarray(W_out, dtype=np.float32),
        np.asarray(b_out, dtype=np.float32))
    res = run_bass_kernel_spmd(nc, in_maps, core_ids=list(range(NCORES)),
                               trace=trace)
    acc = np.zeros((BT, C), dtype=np.float64)
    for c in range(NCORES):
        acc += res.results[c]["y"].astype(np.float64)
    acc += np.asarray(b_out, dtype=np.float64)[None, :]
    out = acc.astype(np.float32).reshape(B, T, C)
    return out, res


def kernel(x, W_qkv, b_qkv, W_out, b_out):
    out, _ = run(x, W_qkv, b_qkv, W_out, b_out, trace=False)
    return out
